# revision 53
# baseline (speedup 1.0000x reference)
"""Trainium2 Bass kernel for CausalQKMemoryProjection.

Math (per batch b, position t, flattened dim D = H*Dh = 1024):
  out_t = (M_p @ q_t + sum_j W[t,j]*g[j]*(q_t . k_j) k_j)
          / (NORM_P + sum_j W[t,j]*g[j]*||k_j||^2 + EPS)
  W[t,j] = gamma^(t-j-1) for 1 <= t-j <= CTX (sliding causal window), else 0.

Sharding: 8 cores = (batch b in 0..3) x (sequence half in 0..1); each core
handles 1024 query positions with a 128-position key halo on the left.

Fast path (m_persistent == s*I, the shipped case), all-bf16 I/O:
the host pre-casts q/k to bf16, sends k in BOTH layouts (kT d-major for
MM1, kN j-major for MM2) so the PE does zero transposes, pre-folds the
decay mask with the gates (agm = W_r * g), and folds the normalizer
rec = 1/(NORM_P + sum W*g*||k||^2 + EPS) into the q columns, so the
device never touches norms.  The output is computed transposed (d, t),
stored bf16, and detransposed/upcast on the host.

Per 128-query block qb (software-pipelined by one block):
  MM1T : dotsT(j,t) [128x256] = kT^T qTs  (2x8 accum matmuls, bf16;
         qTs = q * rec).  Emitted in (j, t) orientation: no transposes.
  DVE  : ag = dotsT * agm  (PSUM -> SBUF bf16, mask+gate folded)
  MM2T : outT(d,t) = sum_j kN^T ag + sI^T qTs  (per d-chunk: 2 kN
         matmuls + 1 diagonal qpers matmul, f32 PSUM accumulation;
         everything is already normalized)
  DVE/ACT: plain PSUM -> bf16 copies into a two-block staging tile,
         one 512B-run store DMA per block pair on the ACT queue.

DMA budget/core: 2MB qTs + 2.25MB kT + 2.25MB kN + 0.53MB cst in,
2MB out; 13 load + 4 store DMAs total (HWDGE setup ~0.6us each is
the reason for the coarse pieces).  Input tiles are double-buffered
(bufs=2 pool) so consecutive loop iterations overlap: the next
iteration's loads start while this one computes.
"""

import numpy as np

B, H, S, Dh = 4, 16, 2048, 64
D = H * Dh            # 1024
CTX = 128
GAMMA = 0.95
NORM_P = D * 0.01     # 10.24
EPS = 1e-6

N_CORES = 8
SLOC = S // 2         # 1024 positions per core
HALO = SLOC + CTX     # 1152 key positions per core
P = 128
NQB = SLOC // P       # 8 query blocks
NDC = D // P          # 8 dim chunks
NJB = HALO // P       # 9 key blocks
PC = 256              # load piece width (columns)


def _make_maskT():
    """maskT (128, 256): [r*128+tt] columns, jj rows; r=0 previous block,
    r=1 diagonal block; t - j = tt - jj + 128*(1-r)."""
    m = np.zeros((P, 2 * P), np.float32)
    jj = np.arange(P)[:, None]
    tt = np.arange(P)[None, :]
    for r in (0, 1):
        delta = tt - jj + 128 * (1 - r)
        w = np.where((delta >= 1) & (delta <= CTX),
                     GAMMA ** np.clip(delta - 1, 0, None).astype(np.float32),
                     0.0)
        m[:, r * P:(r + 1) * P] = w
    return m.astype(np.float32)


_COMPILED = {}


def _build_fast(loop_n=1, out_internal=False, mp_scale=0.1):
    import concourse.mybir as mybir
    import concourse.tile as tile
    from concourse import bacc

    F32 = mybir.dt.float32
    BF16 = mybir.dt.bfloat16

    nc = bacc.Bacc("TRN2", target_bir_lowering=False, debug=False,
                   enable_asserts=False, num_devices=N_CORES)
    qT_d = nc.dram_tensor("qT", (D, SLOC), BF16, kind="ExternalInput").ap()
    kT_d = nc.dram_tensor("kT", (D, HALO), BF16, kind="ExternalInput").ap()
    kN_d = nc.dram_tensor("kN", (HALO, D), BF16, kind="ExternalInput").ap()
    # agm tiles (W_r * g; the normalizer rec is folded into qT by the
    # host) packed with s*I for the qpers diagonal matmuls
    cst_d = nc.dram_tensor("cst", (P, NQB * 2 * P + P), BF16,
                           kind="ExternalInput").ap()
    if out_internal:
        out_d = nc.dram_tensor("out_i", (D, SLOC), BF16, kind="Internal").ap()
        dummy_d = nc.dram_tensor("tiny_out", (1, 1), F32,
                                 kind="ExternalOutput").ap()
    else:
        out_d = nc.dram_tensor("out", (D, SLOC), BF16,
                               kind="ExternalOutput").ap()
        dummy_d = None

    def body(tc, pools):
        perst, work, inbuf, ps_dots, ps_out = pools

        cst_sb = inbuf.tile([P, NQB * 2 * P + P], BF16, tag="cst")
        agm_sb = cst_sb[:, 0:NQB * 2 * P]
        si_sb = cst_sb[:, NQB * 2 * P:]

        # persistent tiles: q/k chunk-stacked along the free dim so one
        # 3D-AP DMA can fill a column piece of every chunk at once
        qT_all = inbuf.tile([P, NDC * SLOC], BF16, tag="qT_all")
        kT_all = inbuf.tile([P, NDC * HALO], BF16, tag="kT_all")
        kN_all = inbuf.tile([P, NJB * D], BF16, tag="kN_all")
        qT_sb = [qT_all[:, ci * SLOC:(ci + 1) * SLOC] for ci in range(NDC)]
        kT_sb = [kT_all[:, ci * HALO:(ci + 1) * HALO] for ci in range(NDC)]
        kN_sb = [kN_all[:, jb * D:(jb + 1) * D] for jb in range(NJB)]
        qT_src = qT_d.rearrange("(c p) s -> p c s", p=P)
        kT_src = kT_d.rearrange("(c p) s -> p c s", p=P)
        kN_src = kN_d.rearrange("(j p) d -> p j d", p=P)
        qT_dst = qT_all.rearrange("p (c s) -> p c s", c=NDC)
        kT_dst = kT_all.rearrange("p (c s) -> p c s", c=NDC)
        kN_dst = kN_all.rearrange("p (j d) -> p j d", j=NJB)

        def emit_mm1(qb):
            """dotsT(j, t) for j-blocks qb (r=0) and qb+1 (r=1)."""
            dps = ps_dots.tile([P, 2 * P], F32, tag="dots")
            for r in (0, 1):
                dsl = dps[:, r * P:(r + 1) * P]
                for ci in range(NDC):
                    nc.tensor.matmul(
                        dsl,
                        kT_sb[ci][:, (qb + r) * P:(qb + r + 1) * P],
                        qT_sb[ci][:, qb * P:(qb + 1) * P],
                        start=(ci == 0), stop=(ci == NDC - 1))
            return dps

        out2_cur = [None]

        def emit_rest(qb, dps):
            # two half-width ops so MM2's r=0 matmuls only wait on the
            # first half (Tile deps are AP-range precise)
            ag = work.tile([P, 2 * P], BF16, tag="ag", name="ag")
            for r in (0, 1):
                nc.vector.tensor_mul(
                    ag[:, r * P:(r + 1) * P], dps[:, r * P:(r + 1) * P],
                    agm_sb[:, (qb * 2 + r) * P:(qb * 2 + r + 1) * P])

            # outT(d, t): lhsT = kN chunk (j, d-cols), rhs = ag (j, t);
            # normalizer is pre-folded into agm/qT, so psum is the final
            # output.  qpers lands in the same accumulation via diagonal
            # matmuls: psum += sI^T @ qTs_chunk (qTs carries rec).
            tcols = slice(qb * P, (qb + 1) * P)
            out_ps = ps_out.tile([P, D], F32, tag="out")
            for ci in range(NDC):
                csl = slice(ci * P, (ci + 1) * P)
                for r in (0, 1):
                    nc.tensor.matmul(out_ps[:, csl],
                                     kN_sb[qb + r][:, csl],
                                     ag[:, r * P:(r + 1) * P],
                                     start=(r == 0), stop=False)
                nc.tensor.matmul(out_ps[:, csl], si_sb[:],
                                 qT_sb[ci][:, tcols],
                                 start=False, stop=True)

            # store staging: pure PSUM -> bf16 copies split DVE/ACT.  Two
            # blocks interleave chunk-major in one (128, 2048) tile so
            # store runs are 512B.
            if qb % 2 == 0:
                out2_cur[0] = work.tile([P, 2 * D], BF16, tag="outN",
                                        name="outN")
            ov4 = out2_cur[0][:].rearrange("p (c two t) -> p c two t",
                                           two=2, c=NDC)
            pv3 = out_ps[:].rearrange("p (c t) -> p c t", c=NDC)
            two = qb % 2
            nc.vector.tensor_copy(ov4[:, 0:4, two, :], pv3[:, 0:4, :])
            nc.scalar.copy(ov4[:, 4:8, two, :], pv3[:, 4:8, :])
            if qb % 2 == 1:
                # outT layout: rows = d (chunk-stacked), cols = 2 t-blocks
                dst = out_d[:, (qb - 1) * P:(qb + 1) * P].rearrange(
                    "(c p) t -> p c t", p=P)
                src = out2_cur[0][:].rearrange("p (c t) -> p c t", c=NDC)
                nc.scalar.dma_start(out=dst, in_=src)

        # DMA-paced: small first pieces so block 0 starts early, bigger
        # later pieces to keep the DMA count low; compute for a block is
        # emitted as soon as its operands' loads are issued
        pieces = [
            ((0, 256),     (0, 2),   (0, 256),     (0,)),
            ((256, 512),   (2, 4),   (256, 512),   (1, 2)),
            ((512, 1024),  (4, 8),   (512, 1024),  (3, 4, 5, 6)),
            ((1024, HALO), (8, NJB), None,         (7,)),
        ]
        prev = None
        for pi, (kt_r, kn_r, qt_r, ready) in enumerate(pieces):
            # MM1 operands (kT, qT) first so block 0 starts earliest
            nc.sync.dma_start(out=kT_dst[:, :, kt_r[0]:kt_r[1]],
                              in_=kT_src[:, :, kt_r[0]:kt_r[1]])
            if qt_r is not None:
                nc.sync.dma_start(out=qT_dst[:, :, qt_r[0]:qt_r[1]],
                                  in_=qT_src[:, :, qt_r[0]:qt_r[1]])
            nc.sync.dma_start(out=kN_dst[:, kn_r[0]:kn_r[1], :],
                              in_=kN_src[:, kn_r[0]:kn_r[1], :])
            if pi == 0:
                nc.sync.dma_start(out=cst_sb[:], in_=cst_d[:, :])
            for qb in ready:
                dps = emit_mm1(qb)
                if prev is not None:
                    emit_rest(*prev)
                prev = (qb, dps)
        emit_rest(*prev)

        if dummy_d is not None:
            nc.sync.dma_start(out=dummy_d[:, :], in_=rec_sb[0:1, 0:1])

    with tile.TileContext(nc) as tc:
        with (
            tc.tile_pool(name="perst", bufs=1) as perst,
            tc.tile_pool(name="work", bufs=2) as work,
            tc.tile_pool(name="inbuf", bufs=2) as inbuf,
            tc.tile_pool(name="ps_dots", bufs=2, space="PSUM") as ps_dots,
            tc.tile_pool(name="ps_out", bufs=3, space="PSUM") as ps_out,
        ):
            pools = (perst, work, inbuf, ps_dots, ps_out)
            if loop_n == 1:
                body(tc, pools)
            else:
                import concourse.mybir as _mb
                hints = (_mb.EngineType.PE, _mb.EngineType.DVE,
                         _mb.EngineType.Activation, _mb.EngineType.SP)
                with tc.For_i(0, loop_n, 1, hint_engines=hints):
                    body(tc, pools)
    nc.compile()
    return nc


def _build_full(loop_n=1, out_internal=False):
    """Fallback for a general (non scaled-identity) m_persistent: the
    original fp32r kernel with on-chip transposes and the full mp matmul."""
    import concourse.mybir as mybir
    import concourse.tile as tile
    from concourse import bacc

    F32 = mybir.dt.float32
    F32R = mybir.dt.float32r

    nc = bacc.Bacc("TRN2", target_bir_lowering=False, debug=False,
                   enable_asserts=False, num_devices=N_CORES)
    qT_d = nc.dram_tensor("qT", (D, SLOC), F32R, kind="ExternalInput").ap()
    kT_d = nc.dram_tensor("kT", (D, HALO), F32R, kind="ExternalInput").ap()
    g_d = nc.dram_tensor("gates", (P, NJB), F32, kind="ExternalInput").ap()
    mask_d = nc.dram_tensor("maskT", (P, 2 * P), F32, kind="ExternalInput").ap()
    eye_d = nc.dram_tensor("eye", (P, P), F32, kind="ExternalInput").ap()
    mp_d = nc.dram_tensor("mp", (D, D), F32R, kind="ExternalInput").ap()
    if out_internal:
        out_d = nc.dram_tensor("out_i", (SLOC, D), F32, kind="Internal").ap()
        dummy_d = nc.dram_tensor("tiny_out", (1, 1), F32,
                                 kind="ExternalOutput").ap()
    else:
        out_d = nc.dram_tensor("out", (SLOC, D), F32,
                               kind="ExternalOutput").ap()
        dummy_d = None

    def body(tc, pools):
        perst, work, small, ps_dots, ps_sh, ps_out = pools

        mask_sb = perst.tile([P, 2 * P], F32, tag="mask")
        nc.sync.dma_start(out=mask_sb[:], in_=mask_d[:, :])
        eye_sb = perst.tile([P, P], F32, tag="eye")
        nc.sync.dma_start(out=eye_sb[:], in_=eye_d[:, :])
        gates_sb = perst.tile([P, NJB], F32, tag="gates")
        nc.sync.dma_start(out=gates_sb[:], in_=g_d[:, :])

        mg_sb = {}
        for qb in range(NQB):
            for r in (0, 1):
                t = perst.tile([P, P], F32, tag=f"mg{qb}_{r}")
                nc.vector.tensor_scalar_mul(
                    t[:], mask_sb[:, r * P:(r + 1) * P],
                    gates_sb[:, qb + r:qb + r + 1])
                mg_sb[(qb, r)] = t

        qT_all = perst.tile([P, NDC * SLOC], F32R, tag="qT_all")
        kT_all = perst.tile([P, NDC * HALO], F32R, tag="kT_all")
        qT_sb = [qT_all[:, ci * SLOC:(ci + 1) * SLOC] for ci in range(NDC)]
        kT_sb = [kT_all[:, ci * HALO:(ci + 1) * HALO] for ci in range(NDC)]
        kN_sb = [perst.tile([P, D], F32R, tag=f"kN{jb}", name=f"kN{jb}")
                 for jb in range(NJB)]
        gknsq_sb = perst.tile([P, NJB], F32, tag="gknsq")
        qT_src = qT_d.rearrange("(c p) s -> p c s", p=P)
        kT_src = kT_d.rearrange("(c p) s -> p c s", p=P)
        qT_dst = qT_all.rearrange("p (c s) -> p c s", c=NDC)
        kT_dst = kT_all.rearrange("p (c s) -> p c s", c=NDC)
        mp_all = perst.tile([P, NDC * D], F32R, tag="mp_all")
        mp_sb = [mp_all[:, ci * D:(ci + 1) * D] for ci in range(NDC)]
        dots_all = ps_dots.tile([P, 512], F32, tag="dots_all")

        cp = [0]

        def emit_kn(jb):
            for half in (0, 1):
                stage = ps_sh.tile([P, 512], F32, tag="at")
                for u in range(4):
                    ci = half * 4 + u
                    nc.tensor.transpose(
                        stage[:, u * P:(u + 1) * P],
                        kT_sb[ci][:, jb * P:(jb + 1) * P].bitcast(F32),
                        eye_sb[:])
                dst = kN_sb[jb][:, half * 512:(half + 1) * 512]
                if cp[0] % 2 == 0:
                    nc.vector.tensor_copy(dst, stage[:])
                else:
                    nc.scalar.copy(dst, stage[:])
                cp[0] += 1
            sq = work.tile([P, D], F32, tag="sq_scratch")
            col = small.tile([P, 1], F32, tag="knsq_col")
            if jb % 2 == 0:
                nc.scalar.activation(sq[:], kN_sb[jb][:].bitcast(F32),
                                     mybir.ActivationFunctionType.Square,
                                     accum_out=col[:])
            else:
                nc.vector.scalar_tensor_tensor(
                    sq[:], kN_sb[jb][:].bitcast(F32), 1.0,
                    kN_sb[jb][:].bitcast(F32),
                    op0=mybir.AluOpType.mult, op1=mybir.AluOpType.mult,
                    accum_out=col[:])
            nc.vector.tensor_mul(gknsq_sb[:, jb:jb + 1], col[:],
                                 gates_sb[:, jb:jb + 1])

        def emit_mm1(qb):
            dsl = dots_all[:, (qb % 2) * 256:(qb % 2) * 256 + 256]
            for ci in range(NDC):
                nc.tensor.matmul(
                    dsl,
                    qT_sb[ci][:, qb * P:(qb + 1) * P],
                    kT_sb[ci][:, qb * P:qb * P + 2 * P],
                    start=(ci == 0), stop=(ci == NDC - 1))
            return dsl

        def emit_rest(qb, dots_ps):
            dots_sb = work.tile([P, 2 * P], F32, tag="dots_sb")
            nc.vector.tensor_copy(dots_sb[:], dots_ps)

            at_ps = ps_sh.tile([P, 264], F32, tag="at")
            for r in (0, 1):
                nc.tensor.transpose(at_ps[:, r * P:(r + 1) * P],
                                    dots_sb[:, r * P:(r + 1) * P], eye_sb[:])
            for r in (0, 1):
                nc.tensor.matmul(at_ps[:, 256:257],
                                 mask_sb[:, r * P:(r + 1) * P],
                                 gknsq_sb[:, qb + r:qb + r + 1],
                                 start=(r == 0), stop=(r == 1))
            ag_sb = []
            for r in (0, 1):
                t = work.tile([P, P], F32R, tag=f"ag{r}", name=f"ag{r}")
                nc.vector.tensor_mul(t[:], at_ps[:, r * P:(r + 1) * P],
                                     mg_sb[(qb, r)][:])
                ag_sb.append(t)
            rec = small.tile([P, 1], F32, tag="rec")
            nc.vector.tensor_scalar_add(rec[:], at_ps[:, 256:257],
                                        NORM_P + EPS)
            rec2 = small.tile([P, 1], F32, tag="rec2")
            nc.vector.reciprocal(rec2[:], rec[:])

            out_ps = ps_out.tile([P, D], F32, tag="out")
            for h in (0, 1):
                sl = slice(h * 512, (h + 1) * 512)
                for r in (0, 1):
                    nc.tensor.matmul(out_ps[:, sl], ag_sb[r][:],
                                     kN_sb[qb + r][:, sl],
                                     start=(r == 0), stop=False)
                for ci in range(NDC):
                    nc.tensor.matmul(
                        out_ps[:, sl],
                        qT_sb[ci][:, qb * P:(qb + 1) * P],
                        mp_sb[ci][:, sl],
                        start=False, stop=(ci == NDC - 1))

            outN = work.tile([P, D], F32, tag="outN")
            nc.vector.tensor_scalar_mul(outN[:, 0:512], out_ps[:, 0:512],
                                        rec2[:])
            nc.scalar.mul(outN[:, 512:1024], out_ps[:, 512:1024], rec2[:])
            nc.sync.dma_start(out=out_d[qb * P:(qb + 1) * P, :], in_=outN[:])

        nc.sync.dma_start(
            out=mp_all.rearrange("p (c s) -> p c s", c=NDC)[:, :, :],
            in_=mp_d.rearrange("(c p) s -> p c s", p=P)[:, :, :])
        NKP = HALO // PC + (1 if HALO % PC else 0)
        NQP = SLOC // PC
        prev = None
        for p in range(NKP):
            klo, khi = p * PC, min((p + 1) * PC, HALO)
            nc.sync.dma_start(out=kT_dst[:, :, klo:khi],
                              in_=kT_src[:, :, klo:khi])
            if p < NQP:
                qlo, qhi = p * PC, (p + 1) * PC
                nc.sync.dma_start(out=qT_dst[:, :, qlo:qhi],
                                  in_=qT_src[:, :, qlo:qhi])
            bpp = PC // P
            for jb in range(p * bpp, min((p + 1) * bpp, NJB)):
                emit_kn(jb)
            ready = [qb for qb in range(NQB)
                     if max(qb // bpp, (qb + 1) // bpp) == p]
            for qb in ready:
                dsl = emit_mm1(qb)
                if prev is not None:
                    emit_rest(*prev)
                prev = (qb, dsl)
        emit_rest(*prev)

        if dummy_d is not None:
            nc.sync.dma_start(out=dummy_d[:, :], in_=gates_sb[0:1, 0:1])

    with tile.TileContext(nc) as tc:
        with (
            tc.tile_pool(name="perst", bufs=1) as perst,
            tc.tile_pool(name="work", bufs=2) as work,
            tc.tile_pool(name="small", bufs=2) as small,
            tc.tile_pool(name="ps_dots", bufs=1, space="PSUM") as ps_dots,
            tc.tile_pool(name="ps_sh", bufs=3, space="PSUM") as ps_sh,
            tc.tile_pool(name="ps_out", bufs=2, space="PSUM") as ps_out,
        ):
            pools = (perst, work, small, ps_dots, ps_sh, ps_out)
            if loop_n == 1:
                body(tc, pools)
            else:
                import concourse.mybir as _mb
                hints = (_mb.EngineType.PE, _mb.EngineType.DVE,
                         _mb.EngineType.Activation, _mb.EngineType.SP)
                with tc.For_i(0, loop_n, 1, hint_engines=hints):
                    body(tc, pools)
    nc.compile()
    return nc


def _get_compiled(loop_n=1, mp_mode="fast", out_internal=False, mp_scale=0.1):
    key = (loop_n, mp_mode, out_internal, mp_scale)
    if key not in _COMPILED:
        if mp_mode == "fast":
            _COMPILED[key] = _build_fast(loop_n, out_internal, mp_scale)
        else:
            _COMPILED[key] = _build_full(loop_n, out_internal)
    return _COMPILED[key]


def _mp_scaled_identity(mp):
    """Return scale s if m_persistent == s * I (exactly), else None."""
    mp = np.asarray(mp)
    if mp.shape != (D, D):
        return None
    s = float(mp[0, 0])
    dg = np.diagonal(mp)
    if not np.all(dg == s):
        return None
    if np.count_nonzero(mp) != np.count_nonzero(dg):
        return None
    return s


def _norm_rec(g, knsq):
    """rec[b, t] = 1/(NORM_P + sum_{d=1..CTX} gamma^(d-1) g[t-d] knsq[t-d] + EPS)."""
    gk = (g[:, :, 0].astype(np.float64) * knsq.astype(np.float64))  # (B, S)
    ker = np.zeros(CTX + 1, np.float64)
    ker[1:] = GAMMA ** np.arange(CTX, dtype=np.float64)
    norm = np.empty_like(gk)
    for b in range(B):
        norm[b] = np.convolve(gk[b], ker)[:S]
    return (1.0 / (norm + NORM_P + EPS)).astype(np.float32)


def _shard_inputs(q, k, gamma_gates, m_persistent):
    """Build the 8 per-core input maps (host-side layout marshaling only)."""
    import ml_dtypes

    BF16 = ml_dtypes.bfloat16
    q = np.asarray(q, np.float32)
    k = np.asarray(k, np.float32)
    g = np.asarray(gamma_gates, np.float32)
    mp = np.ascontiguousarray(np.asarray(m_persistent, np.float32))
    maskT = _make_maskT()
    s = _mp_scaled_identity(mp)

    if s is None:
        return _shard_inputs_full(q, k, g, mp, maskT)

    knsq = np.einsum('bhsd,bhsd->bs', k, k)
    rec = _norm_rec(g, knsq)                                # (B, S)

    in_maps = []
    for c in range(N_CORES):
        b, half = divmod(c, 2)
        t0 = half * SLOC
        qb_ = q[b][:, t0:t0 + SLOC, :]                    # (H, SLOC, Dh)
        rloc = rec[b, t0:t0 + SLOC]                       # (SLOC,)
        qT = (qb_.transpose(0, 2, 1).reshape(D, SLOC)
              * rloc[None, :]).astype(BF16)               # rec folded into q
        lo = t0 - CTX
        if lo < 0:
            kh = np.concatenate(
                [np.zeros((H, CTX, Dh), np.float32), k[b][:, :t0 + SLOC, :]],
                axis=1)
            gh = np.concatenate(
                [np.zeros((CTX, 1), np.float32), g[b][:t0 + SLOC, :]], axis=0)
        else:
            kh = k[b][:, lo:t0 + SLOC, :]
            gh = g[b][lo:t0 + SLOC, :]
        kT = np.ascontiguousarray(
            kh.transpose(0, 2, 1).reshape(D, HALO)).astype(BF16)
        kN = np.ascontiguousarray(
            kh.transpose(1, 0, 2).reshape(HALO, D)).astype(BF16)
        # agm[(qb, r)] = maskT_r * g[j-block qb+r]  (j on partitions)
        cst = np.empty((P, NQB * 2 * P + P), np.float32)
        for qb in range(NQB):
            for r in (0, 1):
                col = gh[(qb + r) * P:(qb + r + 1) * P, 0][:, None]
                cst[:, (qb * 2 + r) * P:(qb * 2 + r + 1) * P] = \
                    maskT[:, r * P:(r + 1) * P] * col
        cst[:, NQB * 2 * P:] = s * np.eye(P, dtype=np.float32)
        in_maps.append({
            "qT": qT, "kT": kT, "kN": kN,
            "cst": cst.astype(BF16),
        })
    return in_maps, "fast"


def _shard_inputs_full(q, k, g, mp, maskT):
    eye = np.eye(P, dtype=np.float32)
    in_maps = []
    for c in range(N_CORES):
        b, half = divmod(c, 2)
        t0 = half * SLOC
        qb_ = q[b][:, t0:t0 + SLOC, :]
        qT = np.ascontiguousarray(qb_.transpose(0, 2, 1).reshape(D, SLOC))
        lo = t0 - CTX
        if lo < 0:
            kh = np.concatenate(
                [np.zeros((H, CTX, Dh), np.float32), k[b][:, :t0 + SLOC, :]],
                axis=1)
            gh = np.concatenate(
                [np.zeros((CTX, 1), np.float32), g[b][:t0 + SLOC, :]], axis=0)
        else:
            kh = k[b][:, lo:t0 + SLOC, :]
            gh = g[b][lo:t0 + SLOC, :]
        kT = np.ascontiguousarray(kh.transpose(0, 2, 1).reshape(D, HALO))
        in_maps.append({
            "qT": qT, "kT": kT,
            "gates": np.ascontiguousarray(gh.reshape(NJB, P).T, np.float32),
            "maskT": maskT, "eye": eye, "mp": mp,
        })
    return in_maps, "full"


def kernel(q, k, gamma_gates, m_persistent):
    from concourse.bass_utils import run_bass_kernel_spmd

    in_maps, mp_mode = _shard_inputs(q, k, gamma_gates, m_persistent)
    s = _mp_scaled_identity(np.asarray(m_persistent, np.float32))
    nc = _get_compiled(1, mp_mode,
                       mp_scale=(s if s is not None else 0.1))
    res = run_bass_kernel_spmd(nc, in_maps, core_ids=list(range(N_CORES)))

    out = np.empty((B, H, S, Dh), np.float32)
    for c in range(N_CORES):
        b, half = divmod(c, 2)
        t0 = half * SLOC
        oc = np.asarray(res.results[c]["out"], dtype=np.float32)
        if mp_mode == "fast":                              # (D, SLOC)
            out[b, :, t0:t0 + SLOC, :] = \
                oc.reshape(H, Dh, SLOC).transpose(0, 2, 1)
        else:                                              # (SLOC, D)
            out[b, :, t0:t0 + SLOC, :] = \
                oc.reshape(SLOC, H, Dh).transpose(1, 0, 2)
    return out
